# revision 14
# baseline (speedup 1.0000x reference)
"""Multi-head self-attention (B=4, L=2048, D=1024, H=16, RoPE, causal) on 8
Trainium2 NeuronCores.

Sharding: data-parallel over batch (4) x tensor-parallel over head groups (2).
Core i handles batch i//2, heads 8*(i%2) .. 8*(i%2)+8.  Each core computes its
QKV projection slice, RoPE, causal attention for its 8 heads, and a partial
output projection over its 512 d-columns; the host sums the two partials per
batch.

On-core dataflow (per core, all matmul operands bf16, psum/softmax fp32):
  qkT[e,l] = Wqk_sub @ x^T       (e = 8 q-heads then 8 k-heads, dh-major)
  rope on qkT rows (pair-swap via stream_shuffle + cos/sin tables)
  V[l,e]   = x @ Wv_sub^T, stored augmented per head pair p:
     A-seg (65 cols):  [V_A | 1]           -> AV out rows 0..63 = O^T_A,
                                              row 64 = rowsum(P_A) = s_A
     B-seg (128 cols): [1*33 | 0*31 | V_B] -> AV out rows 0..32 = s_B,
                                              rows 64..127 = O^T_B
  per head-pair, per 512-q round:
    S^T[k,q] = K^T Q  (row-paired K=64 matmuls, tile_position (0,0)/(64,0))
    P^T = exp(S^T/8) with causal masking (valid-range exp + tri-mask)
    O^T/s via the augmented-V matmuls above (no separate colsum matmuls)
    norm: recip(s) on DVE, Dekker hi+lo bf16 split (ACT/Pool),
          selector-matmul partition broadcast on PE, DVE multiply
  y[l,e] partial = O^T.T @ Wo_sub  (lhsT = O^T chunks)

Schedule: QKV chunk / V-tile / output-projection PE work is interleaved with
the attention rounds so the tensor engine stays busy while the scalar engine
(exp) catches up.  Rounds run pairs {0,1} first (with chunks 2,6,3,7 emitted
as filler), then pairs {2,3} (with output projection as filler).  Inputs are
loaded with batched multi-tile DMA descriptors (SP issue rate is ~650ns per
DMA instruction, so instruction count matters).
"""
import sys
sys.path.insert(0, "/opt/trn_rl_repo")

import numpy as np
import ml_dtypes

B, L, D, H = 4, 2048, 1024, 16
DH = D // H  # 64
THETA = 100000.0
NCORES = 8
BF = ml_dtypes.bfloat16

# vaug column layout (per 128-row L-tile): 4 A-segs of 65, then 4 B-segs of 128
VA_OFF = lambda p: 65 * p
VB_OFF = lambda p: 260 + 128 * p
VAUG_COLS = 772

_built = None


def _rope_tables():
    # [128, L] rows = 2 stacked heads' dh (64 each), identical per head.
    pos = np.arange(L, dtype=np.float32)
    inv_freq = (1.0 / THETA ** (np.arange(0, DH, 2, dtype=np.float32) / DH))
    ang = pos[None, :] * inv_freq[:, None]              # [32, L]
    cos = np.cos(ang)                                    # [32, L]
    sin = np.sin(ang)
    cos2 = np.repeat(cos, 2, axis=0)                     # rows 2p,2p+1 = cos_p
    sin2 = np.empty((DH, L), np.float32)
    sin2[0::2] = -sin
    sin2[1::2] = sin
    return (np.concatenate([cos2, cos2], 0).astype(BF),
            np.concatenate([sin2, sin2], 0).astype(BF))


def _build():
    import concourse.mybir as mybir
    import concourse.tile as tile
    from concourse import bacc

    FP32 = mybir.dt.float32
    BF16 = mybir.dt.bfloat16
    MUL = mybir.AluOpType.mult
    ADD = mybir.AluOpType.add
    SUB = mybir.AluOpType.subtract
    EXP = mybir.ActivationFunctionType.Exp
    SWAP_MASK = [i ^ 1 for i in range(32)]

    nc = bacc.Bacc(None, target_bir_lowering=False)
    # DRAM parameters (per-core shapes; host prepares layouts)
    xt_d = nc.declare_dram_parameter("xt", [128, 8, L], BF16, False)       # [p, dchunk, l]
    wqk_d = nc.declare_dram_parameter("wqk", [8, 128, 1024], BF16, False)  # [echunk, d, dchunk*e]
    wv_d = nc.declare_dram_parameter("wv", [128, 4096], BF16, False)       # [d, dchunk*e_v]
    wo_d = nc.declare_dram_parameter("wo", [128, 4096], BF16, False)       # [d, (dc*2+eh)*e]
    cos_d = nc.declare_dram_parameter("cos2", [128, L], BF16, False)
    sin_d = nc.declare_dram_parameter("sin2", [128, L], BF16, False)
    tri_d = nc.declare_dram_parameter("trimask", [128, 128], BF16, False)
    sel_d = nc.declare_dram_parameter("sel", [128, 128], BF16, False)
    y_d = nc.declare_dram_parameter("y", [L, D], FP32, True)

    with tile.TileContext(nc) as tc:
        import contextlib
        ctx = contextlib.ExitStack()
        with ctx:
            # ---- resident SBUF pools (bufs=1: one slot per tag) ----
            res = ctx.enter_context(tc.tile_pool(name="res", bufs=1))
            # streamed-weight + working pools
            wq_pool = ctx.enter_context(tc.tile_pool(name="wqk", bufs=4))
            rope_pool = ctx.enter_context(tc.tile_pool(name="rope", bufs=3))
            pt_pool = ctx.enter_context(tc.tile_pool(name="pt", bufs=6))
            rec_pool = ctx.enter_context(tc.tile_pool(name="rec", bufs=2))
            y_pool = ctx.enter_context(tc.tile_pool(name="yt", bufs=4))

            xt_all = res.tile([128, 8 * L], BF16, tag="xt", name="xt")
            xt = [xt_all[:, L * d:L * (d + 1)] for d in range(8)]
            qkr = [res.tile([128, L], BF16, tag=f"qkr{c}", name=f"qkr{c}") for c in range(8)]
            vsb = [res.tile([128, VAUG_COLS], BF16, tag=f"v{t}", name=f"v{t}")
                   for t in range(16)]
            wv_all = res.tile([128, 4096], BF16, tag="wv", name="wv")
            wv_sb = [wv_all[:, 512 * d:512 * (d + 1)] for d in range(8)]
            wo_all = res.tile([128, 4096], BF16, tag="wo", name="wo")
            wo_sb = [wo_all[:, 512 * i:512 * (i + 1)] for i in range(8)]
            cos_sb = res.tile([128, L], BF16, tag="cos")
            sin_sb = res.tile([128, L], BF16, tag="sin")
            tri_sb = res.tile([128, 128], BF16, tag="tri")
            sel_sb = res.tile([128, 128], BF16, tag="sel")
            ot = [res.tile([128, L], BF16, tag=f"ot{p}", name=f"ot{p}") for p in range(4)]

            def load_w(c):
                w = wq_pool.tile([128, 1024], BF16, tag="w", name=f"w_{c}")
                nc.sync.dma_start(out=w, in_=wqk_d[c])
                return w

            def xt3(csl):
                return xt_all.rearrange("p (d l) -> p d l", d=8)[:, :, csl]

            # ---- input DMAs: batched, ordered for earliest compute ----
            w0 = load_w(0)
            b0 = slice(0, 512)
            nc.sync.dma_start(out=xt3(b0)[:, 0:4], in_=xt_d[:, 0:4, b0])
            nc.sync.dma_start(out=xt3(b0)[:, 4:8], in_=xt_d[:, 4:8, b0])
            w4 = load_w(4)
            nc.sync.dma_start(out=cos_sb[:, b0], in_=cos_d[:, b0])
            nc.sync.dma_start(out=sin_sb[:, b0], in_=sin_d[:, b0])
            for b4 in range(1, 4):
                csl = slice(512 * b4, 512 * b4 + 512)
                nc.sync.dma_start(out=xt3(csl)[:, 0:4], in_=xt_d[:, 0:4, csl])
                nc.sync.dma_start(out=xt3(csl)[:, 4:8], in_=xt_d[:, 4:8, csl])
                nc.sync.dma_start(out=cos_sb[:, csl], in_=cos_d[:, csl])
                nc.sync.dma_start(out=sin_sb[:, csl], in_=sin_d[:, csl])
                if b4 == 1:
                    nc.sync.dma_start(out=wv_all[:, 0:2048], in_=wv_d[:, 0:2048])
                    nc.sync.dma_start(out=wv_all[:, 2048:4096],
                                      in_=wv_d[:, 2048:4096])
                if b4 == 2:
                    nc.sync.dma_start(out=tri_sb, in_=tri_d[:, :])
                    nc.sync.dma_start(out=sel_sb, in_=sel_d[:, :])

            # augmented-V constant regions: A ones col, B ones/zeros cols
            for t in range(16):
                va = vsb[t][:, 0:260].rearrange("p (pr c) -> p pr c", pr=4)
                vb = vsb[t][:, 260:772].rearrange("p (pr c) -> p pr c", pr=4)
                nc.vector.memset(va[:, :, 64:65], 1.0)
                nc.vector.memset(vb[:, :, 0:33], 1.0)
                nc.vector.memset(vb[:, :, 33:64], 0.0)

            def emit_qk_block(ps_pool, c, l4, w):
                """QKV projection for qk e-chunk c (128 e-cols), L-block l4,
                followed by RoPE into qkr[c]."""
                lsl = slice(512 * l4, 512 * l4 + 512)
                qkp = ps_pool.tile([128, 512], FP32, tag="qkps")
                for d in range(8):
                    nc.tensor.matmul(qkp, w[:, 128 * d:128 * (d + 1)],
                                     xt[d][:, lsl],
                                     start=(d == 0), stop=(d == 7))
                # rope: qkr[c][:,lsl] = qkp*cos + swap(qkp)*sin
                shf = rope_pool.tile([128, 512], FP32, tag="shf")
                nc.vector.stream_shuffle(shf, qkp, SWAP_MASK)
                t1 = rope_pool.tile([128, 512], FP32, tag="t1")
                nc.vector.tensor_tensor(out=t1, in0=qkp, in1=cos_sb[:, lsl], op=MUL)
                t2 = rope_pool.tile([128, 512], FP32, tag="t2")
                nc.vector.tensor_tensor(out=t2, in0=shf, in1=sin_sb[:, lsl], op=MUL)
                nc.gpsimd.tensor_tensor(out=qkr[c][:, lsl], in0=t1, in1=t2, op=ADD)

            def emit_v_tile(ps_pool, t):
                vp = ps_pool.tile([128, 512], FP32, tag="vps")
                lsl = slice(128 * t, 128 * t + 128)
                for d in range(8):
                    nc.tensor.matmul(vp, xt[d][:, lsl], wv_sb[d],
                                     start=(d == 0), stop=(d == 7))
                # scatter into augmented layout: A heads (even), B heads (odd)
                vp3 = vp[:, 0:512].rearrange("p (pr c) -> p pr c", pr=4)
                va = vsb[t][:, 0:260].rearrange("p (pr c) -> p pr c", pr=4)
                vb = vsb[t][:, 260:772].rearrange("p (pr c) -> p pr c", pr=4)
                nc.scalar.copy(out=va[:, :, 0:64], in_=vp3[:, :, 0:64])
                nc.scalar.copy(out=vb[:, :, 64:128], in_=vp3[:, :, 64:128])

            # =========== phase 1: chunks 0,4,1,5 + all V tiles ============
            st_ctx = tc.tile_pool(name="ps_st", bufs=2, space="PSUM")
            st_ps = st_ctx.__enter__()
            try:
                with tc.tile_pool(name="ps_qk", bufs=2, space="PSUM") as qk_ps, \
                     tc.tile_pool(name="ps_v", bufs=2, space="PSUM") as v_ps:
                    for l4 in range(2):
                        emit_qk_block(qk_ps, 0, l4, w0)
                        emit_qk_block(qk_ps, 4, l4, w4)
                    for t in range(4):
                        emit_v_tile(v_ps, t)
                    for l4 in range(2, 4):
                        emit_qk_block(qk_ps, 0, l4, w0)
                        emit_qk_block(qk_ps, 4, l4, w4)
                    w1 = load_w(1)
                    for t in range(4, 8):
                        emit_v_tile(v_ps, t)
                    w5 = load_w(5)
                    for l4 in range(2):
                        emit_qk_block(qk_ps, 1, l4, w1)
                        emit_qk_block(qk_ps, 5, l4, w5)
                    for t in range(8, 12):
                        emit_v_tile(v_ps, t)
                    for l4 in range(2, 4):
                        emit_qk_block(qk_ps, 1, l4, w1)
                        emit_qk_block(qk_ps, 5, l4, w5)
                    for t in range(12, 16):
                        emit_v_tile(v_ps, t)

                # ============ phase 2: attention + filler =============
                with tc.tile_pool(name="ps_av", bufs=1, space="PSUM") as av_ps, \
                     tc.tile_pool(name="ps_bp", bufs=1, space="PSUM") as bp_ps:
                    pending_n1 = [None]
                    pending_n2 = [None]
                    filler_queue = []

                    def emit_norm1(p, qb0, avA, avB, n):
                        """recip + Dekker split (DVE/ACT/Pool)."""
                        rs = rec_pool.tile([128, 512], FP32, tag="rs",
                                           name=f"rs{n}")
                        hi = rec_pool.tile([128, 512], BF16, tag="hi",
                                           name=f"hi{n}")
                        lo = rec_pool.tile([128, 512], BF16, tag="lo",
                                           name=f"lo{n}")
                        # 1/s_A at row 64, 1/s_B at row 32 (custom-DVE ops
                        # need base partition 0; extra rows are unused)
                        nc.vector.reciprocal_approx_fast(
                            out=rs[0:65, :], in_=avA[0:65, :])
                        nc.vector.reciprocal_approx_fast(
                            out=rs[0:33, :], in_=avB[0:33, :])
                        nc.scalar.copy(out=hi[0:65, :], in_=rs[0:65, :])
                        nc.gpsimd.tensor_tensor(out=lo[0:65, :],
                                                in0=rs[0:65, :],
                                                in1=hi[0:65, :], op=SUB)
                        return hi, lo

                    def emit_norm2(p, qb0, avA, avB, hi, lo, n):
                        """selector broadcast (PE) + stage + multiply (DVE)."""
                        qsl = slice(qb0, qb0 + 512)
                        bp = bp_ps.tile([128, 512], FP32, tag="bp",
                                        name=f"bp{n}")
                        nc.tensor.matmul(bp[0:64, :], sel_sb[0:65, 0:64],
                                         hi[0:65, :], start=True, stop=False,
                                         tile_position=(0, 0),
                                         skip_group_check=True)
                        nc.tensor.matmul(bp[0:64, :], sel_sb[0:65, 0:64],
                                         lo[0:65, :], start=False, stop=True,
                                         tile_position=(0, 0),
                                         skip_group_check=True)
                        nc.tensor.matmul(bp[64:128, :], sel_sb[0:65, 64:128],
                                         hi[0:65, :], start=True, stop=False,
                                         tile_position=(0, 64),
                                         skip_group_check=True)
                        nc.tensor.matmul(bp[64:128, :], sel_sb[0:65, 64:128],
                                         lo[0:65, :], start=False, stop=True,
                                         tile_position=(0, 64),
                                         skip_group_check=True)
                        # one engine may read only one PSUM operand: stage bp
                        rsb = rec_pool.tile([128, 512], FP32, tag="rsb",
                                            name=f"rsb{n}")
                        nc.vector.tensor_copy(out=rsb, in_=bp)
                        nc.vector.tensor_tensor(out=ot[p][0:64, qsl],
                                                in0=avA[0:64, :],
                                                in1=rsb[0:64, :], op=MUL)
                        nc.vector.tensor_tensor(out=ot[p][64:128, qsl],
                                                in0=avB[64:128, :],
                                                in1=rsb[64:128, :], op=MUL)

                    def emit_proj_half(y_ps, t, eh):
                        lsl = slice(128 * t, 128 * t + 128)
                        yp = y_ps.tile([128, 512], FP32, tag="yps")
                        for dc in range(4):
                            nc.tensor.matmul(yp, ot[dc][:, lsl],
                                             wo_sb[dc * 2 + eh],
                                             start=(dc == 0), stop=(dc == 3))
                        yt = y_pool.tile([128, 512], FP32, tag="yt")
                        nc.vector.tensor_copy(out=yt, in_=yp)
                        nc.sync.dma_start(
                            out=y_d[lsl, 512 * eh:512 * eh + 512], in_=yt)

                    def emit_round(jq, p, n, cadence):
                        """One attention round: 512 q (block jq), head pair p.
                        Pops filler_queue every `cadence` ktiles."""
                        qb0 = 512 * jq
                        nk = 4 * (jq + 1)
                        avA = av_ps.tile([128, 512], FP32, tag="avA",
                                         name=f"avA{n}")
                        avB = av_ps.tile([128, 512], FP32, tag="avB",
                                         name=f"avB{n}")
                        qt, kt = qkr[p], qkr[4 + p]
                        SKEW = 3
                        pending = []

                        def emit_sav(k, pt, vs):
                            first, last = (k == 0), (k == nk - 1)
                            isl = slice(vs, 512)
                            bsl = slice(512, 1024 - vs)
                            osl = slice(vs, 512)
                            nc.tensor.matmul(avA[0:65, osl],
                                             vsb[k][:, VA_OFF(p):VA_OFF(p) + 65],
                                             pt[:, isl],
                                             start=first, stop=last,
                                             tile_position=(0, 0),
                                             skip_group_check=True)
                            nc.tensor.matmul(avB[:, osl],
                                             vsb[k][:, VB_OFF(p):VB_OFF(p) + 128],
                                             pt[:, bsl],
                                             start=first, stop=last,
                                             tile_position=(0, 0),
                                             skip_group_check=True)

                        for k in range(nk):
                            kpos = 128 * k
                            vs = max(0, kpos - qb0)
                            st = st_ps.tile([128, 1024], FP32, tag="st")
                            ksl = slice(kpos, kpos + 128)
                            qsl = slice(qb0 + vs, qb0 + 512)
                            # head B's block sits at [512 : 1024-vs] so the
                            # written region [vs : 1024-vs] is contiguous
                            nc.tensor.matmul(st[:, vs:512], kt[0:64, ksl],
                                             qt[0:64, qsl], start=True, stop=True,
                                             tile_position=(0, 0))
                            nc.tensor.matmul(st[:, 512:1024 - vs], kt[64:128, ksl],
                                             qt[64:128, qsl], start=True, stop=True,
                                             tile_position=(64, 0))
                            pt = pt_pool.tile([128, 1024], BF16, tag="pt")
                            nc.scalar.activation(out=pt[:, vs:1024 - vs],
                                                 in_=st[:, vs:1024 - vs],
                                                 func=EXP, scale=0.125)
                            if kpos >= qb0:
                                dsl = slice(vs, vs + 128)
                                dslb = slice(512, 640)
                                nc.vector.tensor_tensor(out=pt[:, dsl],
                                                        in0=pt[:, dsl],
                                                        in1=tri_sb, op=MUL)
                                nc.vector.tensor_tensor(out=pt[:, dslb],
                                                        in0=pt[:, dslb],
                                                        in1=tri_sb, op=MUL)
                            if k == 1 and pending_n1[0] is not None:
                                pending_n2[0] = pending_n1[0]()
                                pending_n1[0] = None
                            elif k == 2 and pending_n2[0] is not None:
                                pending_n2[0]()
                                pending_n2[0] = None
                            if filler_queue and (
                                    (cadence >= 3 and k % cadence == cadence - 1)
                                    or (cadence < 3 and k >= 3)):
                                filler_queue.pop(0)()
                            pending.append((k, pt, vs))
                            if len(pending) > SKEW:
                                emit_sav(*pending.pop(0))
                        for args in pending:
                            emit_sav(*args)

                        def make_pending(p=p, qb0=qb0, avA=avA, avB=avB, n=n):
                            def n1():
                                hi, lo = emit_norm1(p, qb0, avA, avB, n)
                                return (lambda: emit_norm2(p, qb0, avA, avB,
                                                           hi, lo, n))
                            return n1
                        pending_n1[0] = make_pending()

                    def flush_norms():
                        if pending_n1[0] is not None:
                            pending_n2[0] = pending_n1[0]()
                            pending_n1[0] = None
                        if pending_n2[0] is not None:
                            pending_n2[0]()
                            pending_n2[0] = None

                    # ---- phase 2a: pairs {0,1}, filler = chunks 2,6,3,7 ----
                    with tc.tile_pool(name="ps_qk2", bufs=1,
                                      space="PSUM") as qk2_ps:
                        w2 = load_w(2)
                        w6 = load_w(6)
                        wlater = {}
                        nc.sync.dma_start(out=wo_all, in_=wo_d[:, :])

                        def chunk_filler(c, l4, w):
                            def go():
                                ww = w if w is not None else wlater[c]
                                emit_qk_block(qk2_ps, c, l4, ww)
                                # chunk 3/7 weights reuse earlier pool slots:
                                # load only after the last reader of the old
                                # weights has been emitted
                                if c == 2 and l4 == 3:
                                    wlater[3] = load_w(3)
                                if c == 6 and l4 == 3:
                                    wlater[7] = load_w(7)
                            return go

                        for l4 in range(4):
                            filler_queue.append(chunk_filler(2, l4, w2))
                            filler_queue.append(chunk_filler(6, l4, w6))
                        for l4 in range(4):
                            filler_queue.append(chunk_filler(3, l4, None))
                            filler_queue.append(chunk_filler(7, l4, None))

                        n = 0
                        for jq in range(4):
                            for p in (0, 1):
                                emit_round(jq, p, n, cadence=4)
                                n += 1
                        while filler_queue:
                            filler_queue.pop(0)()

                    # ---- phase 2b: pairs {2,3}, filler = output proj ----
                    with tc.tile_pool(name="ps_y", bufs=1,
                                      space="PSUM") as y_ps:
                        def proj_filler(t, eh):
                            def go():
                                emit_proj_half(y_ps, t, eh)
                            return go

                        for jq in range(4):
                            for p in (2, 3):
                                emit_round(jq, p, n, cadence=2)
                                n += 1
                                if p == 3:
                                    for t in range(4 * jq, 4 * jq + 4):
                                        for eh in range(2):
                                            filler_queue.append(
                                                proj_filler(t, eh))
                        flush_norms()
                        while filler_queue:
                            filler_queue.pop(0)()
            finally:
                st_ctx.__exit__(None, None, None)
    nc.compile()
    return nc


def _get_nc():
    global _built
    if _built is None:
        _built = _build()
    return _built


def _in_maps(x, W, Wo):
    x = np.asarray(x, np.float32)
    W = np.asarray(W, np.float32)
    Wo = np.asarray(Wo, np.float32)

    cos2, sin2 = _rope_tables()
    tri = np.zeros((128, 128), np.float32)
    p_idx = np.arange(128)
    tri[p_idx[:, None] <= p_idx[None, :]] = 1.0  # valid: k <= q
    tri = tri.astype(BF)
    sel = np.zeros((128, 128), np.float32)
    sel[64, 0:64] = 1.0    # A: broadcast rs row 64 -> bp[0:64]
    sel[32, 64:128] = 1.0  # B: broadcast rs row 32 -> bp[64:128]
    sel = sel.astype(BF)

    in_maps = []
    for core in range(NCORES):
        b, g = core // 2, core % 2
        xt = np.ascontiguousarray(x[b].T).astype(BF)                # [D, L]
        xt = np.ascontiguousarray(
            xt.reshape(8, 128, L).transpose(1, 0, 2))                # [128, 8, L]
        wq = W[512 * g:512 * g + 512]                                # [512, D]
        wk = W[D + 512 * g:D + 512 * g + 512]
        wv = W[2 * D + 512 * g:2 * D + 512 * g + 512]
        wqk_t = np.ascontiguousarray(
            np.concatenate([wq, wk], 0).T).astype(BF)                # [D, 1024]
        # -> [echunk, d, dchunk*128]
        wqk_t = wqk_t.reshape(8, 128, 8, 128).transpose(2, 1, 0, 3)
        wqk_t = np.ascontiguousarray(wqk_t.reshape(8, 128, 1024))
        wv_t = wv.T.astype(BF).reshape(8, 128, 512).transpose(1, 0, 2)
        wv_t = np.ascontiguousarray(wv_t.reshape(128, 4096))
        wo_t = Wo[:, 512 * g:512 * g + 512].T.astype(BF)             # [512, D]
        wo_t = wo_t.reshape(4, 128, 2, 512).transpose(1, 0, 2, 3)
        wo_t = np.ascontiguousarray(wo_t.reshape(128, 4096))
        in_maps.append({
            "xt": xt, "wqk": wqk_t, "wv": wv_t, "wo": wo_t,
            "cos2": cos2, "sin2": sin2, "trimask": tri, "sel": sel,
        })
    return in_maps


def kernel(x, W, Wo):
    from concourse.bass_utils import run_bass_kernel_spmd

    res = run_bass_kernel_spmd(_get_nc(), _in_maps(x, W, Wo),
                               list(range(NCORES)))
    out = np.empty((B, L, D), np.float32)
    for b in range(B):
        out[b] = res.results[2 * b]["y"] + res.results[2 * b + 1]["y"]
    return out


def _install_ntff_hook_shim():
    """The trimmed repo lacks antenv.axon_hooks; reconstruct it so
    run_bass_kernel_spmd(trace=True) can NTFF-profile through axon."""
    import sys as _sys, types
    if "antenv.axon_hooks" in _sys.modules:
        return
    import antenv  # noqa: F401
    from trn_agent_boot.trn_boot import _ntff_profile_via_ctypes
    hook = _ntff_profile_via_ctypes("/opt/axon/libaxon_pjrt.so")
    mod = types.ModuleType("antenv.axon_hooks")
    mod.set_axon_ntff_profile_hook = lambda h: None
    mod.get_axon_ntff_profile_hook = lambda: hook
    _sys.modules["antenv.axon_hooks"] = mod


def kernel_traced(x, W, Wo, tmpdir=None):
    """Run with NTFF tracing; returns BassKernelResults (trace in tmpdir)."""
    from concourse.bass_utils import run_bass_kernel_spmd

    _install_ntff_hook_shim()
    res = run_bass_kernel_spmd(_get_nc(), _in_maps(x, W, Wo),
                               list(range(NCORES)), trace=True, tmpdir=tmpdir)
    return res.exec_time_ns


# revision 15
# speedup vs baseline: 1.0270x; 1.0270x over previous
"""Multi-head self-attention (B=4, L=2048, D=1024, H=16, RoPE, causal) on 8
Trainium2 NeuronCores.

Sharding: data-parallel over batch (4) x tensor-parallel over head groups (2).
Core i handles batch i//2, heads 8*(i%2) .. 8*(i%2)+8.  Each core computes its
QKV projection slice, RoPE, causal attention for its 8 heads, and a partial
output projection over its 512 d-columns; the host sums the two partials per
batch.

On-core dataflow (per core, all matmul operands bf16, psum/softmax fp32):
  qkT[e,l] = Wqk_sub @ x^T       (e = 8 q-heads then 8 k-heads, dh-major)
  rope on qkT rows (pair-swap via stream_shuffle + cos/sin tables)
  V[l,e]   = x @ Wv_sub^T, stored augmented per head pair p:
     A-seg (65 cols):  [V_A | 1]           -> AV out rows 0..63 = O^T_A,
                                              row 64 = rowsum(P_A) = s_A
     B-seg (128 cols): [1 | 0*63 | V_B]  -> AV out row 0 = s_B,
                                              rows 64..127 = O^T_B
  per head-pair, per 512-q round:
    S^T[k,q] = K^T Q  (row-paired K=64 matmuls, tile_position (0,0)/(64,0))
    P^T = exp(S^T/8) with causal masking (valid-range exp + tri-mask)
    O^T/s via the augmented-V matmuls above (no separate colsum matmuls)
    norm: recip(s) on DVE, Dekker hi+lo bf16 split (ACT/Pool),
          selector-matmul partition broadcast on PE, DVE multiply
  y[l,e] partial = O^T.T @ Wo_sub  (lhsT = O^T chunks)

Schedule: QKV chunk / V-tile / output-projection PE work is interleaved with
the attention rounds so the tensor engine stays busy while the scalar engine
(exp) catches up.  Rounds run pairs {0,1} first (with chunks 2,6,3,7 emitted
as filler), then pairs {2,3} (with output projection as filler).  Inputs are
loaded with batched multi-tile DMA descriptors (SP issue rate is ~650ns per
DMA instruction, so instruction count matters).
"""
import sys
sys.path.insert(0, "/opt/trn_rl_repo")

import numpy as np
import ml_dtypes

B, L, D, H = 4, 2048, 1024, 16
DH = D // H  # 64
THETA = 100000.0
NCORES = 8
BF = ml_dtypes.bfloat16

# vaug column layout (per 128-row L-tile): 4 A-segs of 65, then 4 B-segs of 128
VA_OFF = lambda p: 65 * p
VB_OFF = lambda p: 260 + 128 * p
VAUG_COLS = 772

_built = None


def _rope_tables():
    # [128, L] rows = 2 stacked heads' dh (64 each), identical per head.
    pos = np.arange(L, dtype=np.float32)
    inv_freq = (1.0 / THETA ** (np.arange(0, DH, 2, dtype=np.float32) / DH))
    ang = pos[None, :] * inv_freq[:, None]              # [32, L]
    cos = np.cos(ang)                                    # [32, L]
    sin = np.sin(ang)
    cos2 = np.repeat(cos, 2, axis=0)                     # rows 2p,2p+1 = cos_p
    sin2 = np.empty((DH, L), np.float32)
    sin2[0::2] = -sin
    sin2[1::2] = sin
    return (np.concatenate([cos2, cos2], 0).astype(BF),
            np.concatenate([sin2, sin2], 0).astype(BF))


def _build():
    import concourse.mybir as mybir
    import concourse.tile as tile
    from concourse import bacc

    FP32 = mybir.dt.float32
    BF16 = mybir.dt.bfloat16
    MUL = mybir.AluOpType.mult
    ADD = mybir.AluOpType.add
    SUB = mybir.AluOpType.subtract
    EXP = mybir.ActivationFunctionType.Exp
    SWAP_MASK = [i ^ 1 for i in range(32)]

    nc = bacc.Bacc(None, target_bir_lowering=False)
    # DRAM parameters (per-core shapes; host prepares layouts)
    xt_d = nc.declare_dram_parameter("xt", [128, 8, L], BF16, False)       # [p, dchunk, l]
    wqk_d = nc.declare_dram_parameter("wqk", [8, 128, 1024], BF16, False)  # [echunk, d, dchunk*e]
    wv_d = nc.declare_dram_parameter("wv", [128, 4096], BF16, False)       # [d, dchunk*e_v]
    wo_d = nc.declare_dram_parameter("wo", [128, 4096], BF16, False)       # [d, (dc*2+eh)*e]
    cos_d = nc.declare_dram_parameter("cos2", [128, L], BF16, False)
    sin_d = nc.declare_dram_parameter("sin2", [128, L], BF16, False)
    tri_d = nc.declare_dram_parameter("trimask", [128, 128], BF16, False)
    sel_d = nc.declare_dram_parameter("sel", [128, 128], BF16, False)
    y_d = nc.declare_dram_parameter("y", [L, D], FP32, True)

    with tile.TileContext(nc) as tc:
        import contextlib
        ctx = contextlib.ExitStack()
        with ctx:
            # ---- resident SBUF pools (bufs=1: one slot per tag) ----
            res = ctx.enter_context(tc.tile_pool(name="res", bufs=1))
            # streamed-weight + working pools
            wq_pool = ctx.enter_context(tc.tile_pool(name="wqk", bufs=4))
            rope_pool = ctx.enter_context(tc.tile_pool(name="rope", bufs=3))
            pt_pool = ctx.enter_context(tc.tile_pool(name="pt", bufs=6))
            rec_pool = ctx.enter_context(tc.tile_pool(name="rec", bufs=2))
            y_pool = ctx.enter_context(tc.tile_pool(name="yt", bufs=4))

            xt_all = res.tile([128, 8 * L], BF16, tag="xt", name="xt")
            xt = [xt_all[:, L * d:L * (d + 1)] for d in range(8)]
            qkr = [res.tile([128, L], BF16, tag=f"qkr{c}", name=f"qkr{c}") for c in range(8)]
            vsb = [res.tile([128, VAUG_COLS], BF16, tag=f"v{t}", name=f"v{t}")
                   for t in range(16)]
            wv_all = res.tile([128, 4096], BF16, tag="wv", name="wv")
            wv_sb = [wv_all[:, 512 * d:512 * (d + 1)] for d in range(8)]
            wo_all = res.tile([128, 4096], BF16, tag="wo", name="wo")
            wo_sb = [wo_all[:, 512 * i:512 * (i + 1)] for i in range(8)]
            cos_sb = res.tile([128, L], BF16, tag="cos")
            sin_sb = res.tile([128, L], BF16, tag="sin")
            tri_sb = res.tile([128, 128], BF16, tag="tri")
            sel_sb = res.tile([128, 128], BF16, tag="sel")
            ot = [res.tile([128, L], BF16, tag=f"ot{p}", name=f"ot{p}") for p in range(4)]

            def load_w(c):
                w = wq_pool.tile([128, 1024], BF16, tag="w", name=f"w_{c}")
                nc.sync.dma_start(out=w, in_=wqk_d[c])
                return w

            def xt3(csl):
                return xt_all.rearrange("p (d l) -> p d l", d=8)[:, :, csl]

            # ---- input DMAs: batched, ordered for earliest compute ----
            w0 = load_w(0)
            b0 = slice(0, 512)
            for dd in range(4):
                nc.sync.dma_start(out=xt3(b0)[:, 2 * dd:2 * dd + 2],
                                  in_=xt_d[:, 2 * dd:2 * dd + 2, b0])
            w4 = load_w(4)
            nc.sync.dma_start(out=cos_sb[:, b0], in_=cos_d[:, b0])
            nc.sync.dma_start(out=sin_sb[:, b0], in_=sin_d[:, b0])
            for b4 in range(1, 4):
                csl = slice(512 * b4, 512 * b4 + 512)
                nc.sync.dma_start(out=xt3(csl)[:, 0:4], in_=xt_d[:, 0:4, csl])
                nc.sync.dma_start(out=xt3(csl)[:, 4:8], in_=xt_d[:, 4:8, csl])
                nc.sync.dma_start(out=cos_sb[:, csl], in_=cos_d[:, csl])
                nc.sync.dma_start(out=sin_sb[:, csl], in_=sin_d[:, csl])
                if b4 == 1:
                    nc.sync.dma_start(out=wv_all[:, 0:2048], in_=wv_d[:, 0:2048])
                    nc.sync.dma_start(out=wv_all[:, 2048:4096],
                                      in_=wv_d[:, 2048:4096])
                if b4 == 2:
                    nc.sync.dma_start(out=tri_sb, in_=tri_d[:, :])
                    nc.sync.dma_start(out=sel_sb, in_=sel_d[:, :])

            # augmented-V constant regions: A ones col, B ones/zeros cols
            for t in range(16):
                va = vsb[t][:, 0:260].rearrange("p (pr c) -> p pr c", pr=4)
                vb = vsb[t][:, 260:772].rearrange("p (pr c) -> p pr c", pr=4)
                nc.vector.memset(va[:, :, 64:65], 1.0)
                nc.vector.memset(vb[:, :, 0:1], 1.0)
                nc.vector.memset(vb[:, :, 1:64], 0.0)

            def emit_qk_block(ps_pool, c, l4, w):
                """QKV projection for qk e-chunk c (128 e-cols), L-block l4,
                followed by RoPE into qkr[c]."""
                lsl = slice(512 * l4, 512 * l4 + 512)
                qkp = ps_pool.tile([128, 512], FP32, tag="qkps")
                for d in range(8):
                    nc.tensor.matmul(qkp, w[:, 128 * d:128 * (d + 1)],
                                     xt[d][:, lsl],
                                     start=(d == 0), stop=(d == 7))
                # rope: qkr[c][:,lsl] = qkp*cos + swap(qkp)*sin
                shf = rope_pool.tile([128, 512], FP32, tag="shf")
                nc.vector.stream_shuffle(shf, qkp, SWAP_MASK)
                t1 = rope_pool.tile([128, 512], FP32, tag="t1")
                nc.vector.tensor_tensor(out=t1, in0=qkp, in1=cos_sb[:, lsl], op=MUL)
                t2 = rope_pool.tile([128, 512], FP32, tag="t2")
                nc.vector.tensor_tensor(out=t2, in0=shf, in1=sin_sb[:, lsl], op=MUL)
                nc.gpsimd.tensor_tensor(out=qkr[c][:, lsl], in0=t1, in1=t2, op=ADD)

            def emit_v_tile(ps_pool, t):
                vp = ps_pool.tile([128, 512], FP32, tag="vps")
                lsl = slice(128 * t, 128 * t + 128)
                for d in range(8):
                    nc.tensor.matmul(vp, xt[d][:, lsl], wv_sb[d],
                                     start=(d == 0), stop=(d == 7))
                # scatter into augmented layout: A heads (even), B heads (odd)
                vp3 = vp[:, 0:512].rearrange("p (pr c) -> p pr c", pr=4)
                va = vsb[t][:, 0:260].rearrange("p (pr c) -> p pr c", pr=4)
                vb = vsb[t][:, 260:772].rearrange("p (pr c) -> p pr c", pr=4)
                nc.scalar.copy(out=va[:, :, 0:64], in_=vp3[:, :, 0:64])
                nc.scalar.copy(out=vb[:, :, 64:128], in_=vp3[:, :, 64:128])

            # =========== phase 1: chunks 0,4,1,5 + all V tiles ============
            st_ctx = tc.tile_pool(name="ps_st", bufs=2, space="PSUM")
            st_ps = st_ctx.__enter__()
            try:
                with tc.tile_pool(name="ps_qk", bufs=2, space="PSUM") as qk_ps, \
                     tc.tile_pool(name="ps_v", bufs=2, space="PSUM") as v_ps:
                    for l4 in range(2):
                        emit_qk_block(qk_ps, 0, l4, w0)
                        emit_qk_block(qk_ps, 4, l4, w4)
                    for t in range(4):
                        emit_v_tile(v_ps, t)
                    for l4 in range(2, 4):
                        emit_qk_block(qk_ps, 0, l4, w0)
                        emit_qk_block(qk_ps, 4, l4, w4)
                    w1 = load_w(1)
                    for t in range(4, 8):
                        emit_v_tile(v_ps, t)
                    w5 = load_w(5)
                    for l4 in range(2):
                        emit_qk_block(qk_ps, 1, l4, w1)
                        emit_qk_block(qk_ps, 5, l4, w5)
                    for t in range(8, 12):
                        emit_v_tile(v_ps, t)
                    for l4 in range(2, 4):
                        emit_qk_block(qk_ps, 1, l4, w1)
                        emit_qk_block(qk_ps, 5, l4, w5)
                    for t in range(12, 16):
                        emit_v_tile(v_ps, t)

                # ============ phase 2: attention + filler =============
                with tc.tile_pool(name="ps_av", bufs=1, space="PSUM") as av_ps, \
                     tc.tile_pool(name="ps_bp", bufs=1, space="PSUM") as bp_ps:
                    pending_n1 = [None]
                    pending_n2 = [None]
                    filler_queue = []

                    def emit_norm1(p, qb0, avA, avB, n):
                        """recip + Dekker split (DVE/ACT/Pool)."""
                        rs = rec_pool.tile([128, 512], FP32, tag="rs",
                                           name=f"rs{n}")
                        hi = rec_pool.tile([128, 512], BF16, tag="hi",
                                           name=f"hi{n}")
                        lo = rec_pool.tile([128, 512], BF16, tag="lo",
                                           name=f"lo{n}")
                        # 1/s_A at row 64, 1/s_B at row 32 (custom-DVE ops
                        # need base partition 0; extra rows are unused)
                        nc.vector.reciprocal_approx_fast(
                            out=rs[0:65, :], in_=avA[0:65, :])
                        nc.vector.reciprocal_approx_fast(
                            out=rs[0:1, :], in_=avB[0:1, :])
                        nc.scalar.copy(out=hi[0:65, :], in_=rs[0:65, :])
                        nc.gpsimd.tensor_tensor(out=lo[0:65, :],
                                                in0=rs[0:65, :],
                                                in1=hi[0:65, :], op=SUB)
                        return hi, lo

                    def emit_norm2(p, qb0, avA, avB, hi, lo, n):
                        """selector broadcast (PE) + stage + multiply (DVE)."""
                        qsl = slice(qb0, qb0 + 512)
                        bp = bp_ps.tile([128, 512], FP32, tag="bp",
                                        name=f"bp{n}")
                        nc.tensor.matmul(bp[:, :], sel_sb[0:65, :],
                                         hi[0:65, :], start=True, stop=False,
                                         tile_position=(0, 0),
                                         skip_group_check=True)
                        nc.tensor.matmul(bp[:, :], sel_sb[0:65, :],
                                         lo[0:65, :], start=False, stop=True,
                                         tile_position=(0, 0),
                                         skip_group_check=True)
                        # one engine may read only one PSUM operand: stage bp
                        rsb = rec_pool.tile([128, 512], FP32, tag="rsb",
                                            name=f"rsb{n}")
                        nc.vector.tensor_copy(out=rsb, in_=bp)
                        nc.vector.tensor_tensor(out=ot[p][0:64, qsl],
                                                in0=avA[0:64, :],
                                                in1=rsb[0:64, :], op=MUL)
                        nc.vector.tensor_tensor(out=ot[p][64:128, qsl],
                                                in0=avB[64:128, :],
                                                in1=rsb[64:128, :], op=MUL)

                    def emit_proj_half(y_ps, t, eh):
                        lsl = slice(128 * t, 128 * t + 128)
                        yp = y_ps.tile([128, 512], FP32, tag="yps")
                        for dc in range(4):
                            nc.tensor.matmul(yp, ot[dc][:, lsl],
                                             wo_sb[dc * 2 + eh],
                                             start=(dc == 0), stop=(dc == 3))
                        yt = y_pool.tile([128, 512], FP32, tag="yt")
                        nc.vector.tensor_copy(out=yt, in_=yp)
                        nc.sync.dma_start(
                            out=y_d[lsl, 512 * eh:512 * eh + 512], in_=yt)

                    def emit_round(jq, p, n, cadence):
                        """One attention round: 512 q (block jq), head pair p.
                        Pops filler_queue every `cadence` ktiles."""
                        qb0 = 512 * jq
                        nk = 4 * (jq + 1)
                        avA = av_ps.tile([128, 512], FP32, tag="avA",
                                         name=f"avA{n}")
                        avB = av_ps.tile([128, 512], FP32, tag="avB",
                                         name=f"avB{n}")
                        qt, kt = qkr[p], qkr[4 + p]
                        SKEW = 3
                        pending = []

                        def emit_sav(k, pt, vs):
                            first, last = (k == 0), (k == nk - 1)
                            isl = slice(vs, 512)
                            bsl = slice(512, 1024 - vs)
                            osl = slice(vs, 512)
                            nc.tensor.matmul(avA[0:65, osl],
                                             vsb[k][:, VA_OFF(p):VA_OFF(p) + 65],
                                             pt[:, isl],
                                             start=first, stop=last,
                                             tile_position=(0, 0),
                                             skip_group_check=True)
                            nc.tensor.matmul(avB[:, osl],
                                             vsb[k][:, VB_OFF(p):VB_OFF(p) + 128],
                                             pt[:, bsl],
                                             start=first, stop=last,
                                             tile_position=(0, 0),
                                             skip_group_check=True)

                        for k in range(nk):
                            kpos = 128 * k
                            vs = max(0, kpos - qb0)
                            st = st_ps.tile([128, 1024], FP32, tag="st")
                            ksl = slice(kpos, kpos + 128)
                            qsl = slice(qb0 + vs, qb0 + 512)
                            # head B's block sits at [512 : 1024-vs] so the
                            # written region [vs : 1024-vs] is contiguous
                            nc.tensor.matmul(st[:, vs:512], kt[0:64, ksl],
                                             qt[0:64, qsl], start=True, stop=True,
                                             tile_position=(0, 0))
                            nc.tensor.matmul(st[:, 512:1024 - vs], kt[64:128, ksl],
                                             qt[64:128, qsl], start=True, stop=True,
                                             tile_position=(64, 0))
                            pt = pt_pool.tile([128, 1024], BF16, tag="pt")
                            nc.scalar.activation(out=pt[:, vs:1024 - vs],
                                                 in_=st[:, vs:1024 - vs],
                                                 func=EXP, scale=0.125)
                            if kpos >= qb0:
                                dsl = slice(vs, vs + 128)
                                dslb = slice(512, 640)
                                nc.vector.tensor_tensor(out=pt[:, dsl],
                                                        in0=pt[:, dsl],
                                                        in1=tri_sb, op=MUL)
                                nc.vector.tensor_tensor(out=pt[:, dslb],
                                                        in0=pt[:, dslb],
                                                        in1=tri_sb, op=MUL)
                            if k == 1 and pending_n1[0] is not None:
                                pending_n2[0] = pending_n1[0]()
                                pending_n1[0] = None
                            elif k == 2 and pending_n2[0] is not None:
                                pending_n2[0]()
                                pending_n2[0] = None
                            if filler_queue and (
                                    (cadence >= 3 and k % cadence == cadence - 1)
                                    or (cadence < 3 and k >= 3)):
                                filler_queue.pop(0)()
                            pending.append((k, pt, vs))
                            if len(pending) > SKEW:
                                emit_sav(*pending.pop(0))
                        for args in pending:
                            emit_sav(*args)

                        def make_pending(p=p, qb0=qb0, avA=avA, avB=avB, n=n):
                            def n1():
                                hi, lo = emit_norm1(p, qb0, avA, avB, n)
                                return (lambda: emit_norm2(p, qb0, avA, avB,
                                                           hi, lo, n))
                            return n1
                        pending_n1[0] = make_pending()

                    def flush_norms():
                        if pending_n1[0] is not None:
                            pending_n2[0] = pending_n1[0]()
                            pending_n1[0] = None
                        if pending_n2[0] is not None:
                            pending_n2[0]()
                            pending_n2[0] = None

                    # ---- phase 2a: pairs {0,1}, filler = chunks 2,6,3,7 ----
                    with tc.tile_pool(name="ps_qk2", bufs=1,
                                      space="PSUM") as qk2_ps:
                        w2 = load_w(2)
                        w6 = load_w(6)
                        wlater = {}
                        nc.sync.dma_start(out=wo_all, in_=wo_d[:, :])

                        def chunk_filler(c, l4, w):
                            def go():
                                ww = w if w is not None else wlater[c]
                                emit_qk_block(qk2_ps, c, l4, ww)
                                # chunk 3/7 weights reuse earlier pool slots:
                                # load only after the last reader of the old
                                # weights has been emitted
                                if c == 2 and l4 == 3:
                                    wlater[3] = load_w(3)
                                if c == 6 and l4 == 3:
                                    wlater[7] = load_w(7)
                            return go

                        for l4 in range(4):
                            filler_queue.append(chunk_filler(2, l4, w2))
                            filler_queue.append(chunk_filler(6, l4, w6))
                        for l4 in range(4):
                            filler_queue.append(chunk_filler(3, l4, None))
                            filler_queue.append(chunk_filler(7, l4, None))

                        n = 0
                        for jq in range(4):
                            for p in (0, 1):
                                emit_round(jq, p, n, cadence=4)
                                n += 1
                        while filler_queue:
                            filler_queue.pop(0)()

                    # ---- phase 2b: pairs {2,3}, filler = output proj ----
                    with tc.tile_pool(name="ps_y", bufs=1,
                                      space="PSUM") as y_ps:
                        def proj_filler(t, eh):
                            def go():
                                emit_proj_half(y_ps, t, eh)
                            return go

                        for jq in range(4):
                            for p in (2, 3):
                                emit_round(jq, p, n, cadence=2)
                                n += 1
                                if p == 3:
                                    for t in range(4 * jq, 4 * jq + 4):
                                        for eh in range(2):
                                            filler_queue.append(
                                                proj_filler(t, eh))
                        flush_norms()
                        while filler_queue:
                            filler_queue.pop(0)()
            finally:
                st_ctx.__exit__(None, None, None)
    nc.compile()
    return nc


def _get_nc():
    global _built
    if _built is None:
        _built = _build()
    return _built


def _in_maps(x, W, Wo):
    x = np.asarray(x, np.float32)
    W = np.asarray(W, np.float32)
    Wo = np.asarray(Wo, np.float32)

    cos2, sin2 = _rope_tables()
    tri = np.zeros((128, 128), np.float32)
    p_idx = np.arange(128)
    tri[p_idx[:, None] <= p_idx[None, :]] = 1.0  # valid: k <= q
    tri = tri.astype(BF)
    sel = np.zeros((128, 128), np.float32)
    sel[64, 0:64] = 1.0    # A: broadcast rs row 64 -> bp[0:64]
    sel[0, 64:128] = 1.0   # B: broadcast rs row 0 -> bp[64:128]
    sel = sel.astype(BF)

    in_maps = []
    for core in range(NCORES):
        b, g = core // 2, core % 2
        xt = np.ascontiguousarray(x[b].T).astype(BF)                # [D, L]
        xt = np.ascontiguousarray(
            xt.reshape(8, 128, L).transpose(1, 0, 2))                # [128, 8, L]
        wq = W[512 * g:512 * g + 512]                                # [512, D]
        wk = W[D + 512 * g:D + 512 * g + 512]
        wv = W[2 * D + 512 * g:2 * D + 512 * g + 512]
        wqk_t = np.ascontiguousarray(
            np.concatenate([wq, wk], 0).T).astype(BF)                # [D, 1024]
        # -> [echunk, d, dchunk*128]
        wqk_t = wqk_t.reshape(8, 128, 8, 128).transpose(2, 1, 0, 3)
        wqk_t = np.ascontiguousarray(wqk_t.reshape(8, 128, 1024))
        wv_t = wv.T.astype(BF).reshape(8, 128, 512).transpose(1, 0, 2)
        wv_t = np.ascontiguousarray(wv_t.reshape(128, 4096))
        wo_t = Wo[:, 512 * g:512 * g + 512].T.astype(BF)             # [512, D]
        wo_t = wo_t.reshape(4, 128, 2, 512).transpose(1, 0, 2, 3)
        wo_t = np.ascontiguousarray(wo_t.reshape(128, 4096))
        in_maps.append({
            "xt": xt, "wqk": wqk_t, "wv": wv_t, "wo": wo_t,
            "cos2": cos2, "sin2": sin2, "trimask": tri, "sel": sel,
        })
    return in_maps


def kernel(x, W, Wo):
    from concourse.bass_utils import run_bass_kernel_spmd

    res = run_bass_kernel_spmd(_get_nc(), _in_maps(x, W, Wo),
                               list(range(NCORES)))
    out = np.empty((B, L, D), np.float32)
    for b in range(B):
        out[b] = res.results[2 * b]["y"] + res.results[2 * b + 1]["y"]
    return out


def _install_ntff_hook_shim():
    """The trimmed repo lacks antenv.axon_hooks; reconstruct it so
    run_bass_kernel_spmd(trace=True) can NTFF-profile through axon."""
    import sys as _sys, types
    if "antenv.axon_hooks" in _sys.modules:
        return
    import antenv  # noqa: F401
    from trn_agent_boot.trn_boot import _ntff_profile_via_ctypes
    hook = _ntff_profile_via_ctypes("/opt/axon/libaxon_pjrt.so")
    mod = types.ModuleType("antenv.axon_hooks")
    mod.set_axon_ntff_profile_hook = lambda h: None
    mod.get_axon_ntff_profile_hook = lambda: hook
    _sys.modules["antenv.axon_hooks"] = mod


def kernel_traced(x, W, Wo, tmpdir=None):
    """Run with NTFF tracing; returns BassKernelResults (trace in tmpdir)."""
    from concourse.bass_utils import run_bass_kernel_spmd

    _install_ntff_hook_shim()
    res = run_bass_kernel_spmd(_get_nc(), _in_maps(x, W, Wo),
                               list(range(NCORES)), trace=True, tmpdir=tmpdir)
    return res.exec_time_ns
